# revision 16
# baseline (speedup 1.0000x reference)
"""MinLSTM Trainium2 kernel (v2: engine-rebalanced).

Math (identical to the log-space reference, in linear space):
    sf = sigmoid(x @ W_f.T + b_f)
    si = sigmoid(x @ W_i.T + b_i)
    zh = x @ W_h.T + b_h
    g  = max(zh + 0.5, sigmoid(zh))           (exact rewrite of log_g)
    aa = sf / (sf + si)                        (normalized forget gate)
    bb = si * g / (sf + si)                    (normalized input contribution)
    h_t = aa_t * h_{t-1} + bb_t                (hardware tensor_tensor_scan)

Sharding: data-parallel over batch B=8, one batch per NeuronCore. Host
pre-transposes x[b] to [D, T] so gate matmuls produce z in [H-partition,
T-free] layout, which the per-partition scan along the free dim needs.

v2 engine assignment (v1 was DVE-bound at ~84% busy):
  - ts = sf+si moved from DVE TensorTensor to the PE: two identity-weight
    bf16 matmuls accumulating into a PSUM tile (PE had ~55% headroom).
  - aa = 1-qq moved from ACT Identity to DVE tensor_scalar (plain TSP has
    the 4x_2p fast mode; ACT was the secondary bottleneck).
  - qq = rr*si stays on Pool (GpSimd TT mult, its only fast op).
  - gg = max(zh+bh+.5, sigmoid(zh+bh)): per-unit choice between one DVE
    stt (no fast mode, 1x) and ACT Identity zp + DVE TT max (2x) -- the
    gg_act knob rebalances ACT vs DVE load.
  - bb = qq*gg: per-unit choice DVE TT (2x) vs Pool TT (bb_pool knob).
  - scans are DVE tensor_tensor_scan (DVE-only op, no fast mode).

Three-stage software pipeline (a: f/i matmuls+sigmoids; a2: ts/rr/qq/aa;
b: h matmul+g+bb+scan+out) keeps the PE queue from head-blocking on the
identity matmuls (ts(k) is emitted after zf/zi(k+1)).

mm="hybrid": f,i gate matmuls in fp8e4 with DoubleRow (2x PE rate,
quarter DMA), h gate in bf16 (g is linear in zh for zh>0, so fp8 x
quantization there would not cancel).
"""

import os
import sys

for _p in ("/opt/trn_rl_repo", "/root/.axon_site/_ro/trn_rl_repo"):
    if os.path.isdir(_p) and _p not in sys.path:
        sys.path.insert(0, _p)

import numpy as np

import concourse.bacc as bacc
import concourse.tile as tile
from concourse import bass_utils, mybir
from concourse.mybir import ActivationFunctionType as AF
from concourse.mybir import AluOpType as ALU

B, T, D, H = 8, 4096, 512, 512
P = 128
KD = D // P       # 4 contraction blocks
HB = H // P       # 4 hidden-partition blocks
TQ = 1024         # matmul/unit width (2 fp32 PSUM banks)
NQ = T // TQ      # 4 quarters
F32 = mybir.dt.float32
BF16 = mybir.dt.bfloat16
FP8 = mybir.dt.float8e4

MM = "hybrid"     # "bf16" | "fp8" | "hybrid" (f,i gates fp8; h gate bf16)
OUT_DT = BF16

_CACHE = {}


def _gate_dtypes(mm):
    """Per-gate matmul dtype: gates (0=f, 1=i, 2=h)."""
    if mm == "bf16":
        return {0: BF16, 1: BF16, 2: BF16}
    if mm == "fp8":
        return {0: FP8, 1: FP8, 2: FP8}
    if mm == "hybrid":
        return {0: FP8, 1: FP8, 2: BF16}
    raise ValueError(mm)


def _spread(n, total=16):
    """Bresenham-spread set of n unit indices out of `total`."""
    if n <= 0:
        return set()
    return {(i * total) // n for i in range(n)}


def _build(n_cores=B, loop_reps=0, mm=MM, out_dt=OUT_DT, ablate=(),
           gg_act=2, bb_pool=8, qq_eng="gpsimd", ts_pe=True,
           lag_a2=1, lag_b=1, bufs=5, hop_bufs=2, tail_units=2,
           z_bufs=3, ts_bufs=1, b2_same=True):
    gdt = _gate_dtypes(mm)
    any8 = any(d == FP8 for d in gdt.values())
    anyb = any(d == BF16 for d in gdt.values())
    nc = bacc.Bacc("TRN2", target_bir_lowering=False, debug=False,
                   num_devices=n_cores)
    g8 = [g for g in range(3) if gdt[g] == FP8]   # fp8 gates
    gb = [g for g in range(3) if gdt[g] == BF16]  # bf16 gates
    # All tensors partition-major so each load is ONE dma_start:
    #   x8 [p, j, i, t]   with d = (2j+i)*128+p (DoubleRow pairs)
    #   xb [p, k, t]      with d = k*128+p
    #   w8 [p, gi, j, i, h] / wb [p, ki, h]  (gi/ki index into g8/gb lists)
    x8_d = (nc.dram_tensor("xT8", [P, 2, 2, T], FP8, kind="ExternalInput")
            if any8 else None)
    xb_d = (nc.dram_tensor("xTb", [P, KD, T], BF16, kind="ExternalInput")
            if anyb else None)
    w8_d = (nc.dram_tensor("wT8", [P, len(g8), 2, 2, H], FP8,
                           kind="ExternalInput") if any8 else None)
    wb_d = (nc.dram_tensor("wTb", [P, len(gb) * KD, H], BF16,
                           kind="ExternalInput") if anyb else None)
    # 4 bias groups packed per partition: [b_f | b_i | b_h | b_h + 0.5]
    bias_d = nc.dram_tensor("biasp", [P, 4 * HB], F32, kind="ExternalInput")
    h0_d = nc.dram_tensor("h0p", [P, HB], F32, kind="ExternalInput")
    id_d = (nc.dram_tensor("identp", [P, P], BF16, kind="ExternalInput")
            if ts_pe else None)
    ht_d = nc.dram_tensor("ht", [H, T], out_dt, kind="ExternalOutput")

    gg_act_set = _spread(gg_act)
    bb_pool_set = _spread(bb_pool)

    with tile.TileContext(nc) as tc:
        with (
            tc.tile_pool(name="xp", bufs=1) as xp,
            tc.tile_pool(name="wp", bufs=1) as wp,
            tc.tile_pool(name="cp", bufs=1) as cp,
            tc.tile_pool(name="ps", bufs=z_bufs, space="PSUM") as ps,
            tc.tile_pool(name="tsp", bufs=ts_bufs, space="PSUM") as tsp,
            tc.tile_pool(name="gp", bufs=bufs) as gp,
            tc.tile_pool(name="ep", bufs=bufs) as ep,
            tc.tile_pool(name="hop", bufs=hop_bufs) as hop,
        ):
            bias = cp.tile([P, 4 * HB], F32, tag="bias")
            h0 = cp.tile([P, HB], F32, tag="h0")
            ident = cp.tile([P, P], BF16, tag="ident") if ts_pe else None
            warm = cp.tile([P, 1], F32, tag="warm")

            def load_consts():
                """Emitted after the first matmul-feeding DMAs: bias is first
                needed by sigma_f at ~5us, ident by the first idmm, h0 by the
                first scan -- none on the startup critical path."""
                nc.sync.dma_start(bias[:], bias_d.ap())
                if ts_pe:
                    nc.sync.dma_start(ident[:], id_d.ap())
                nc.sync.dma_start(h0[:], h0_d.ap())
                # prime the ACT sigmoid table off the critical path
                nc.scalar.activation(warm[:], h0[:, 0:1], AF.Sigmoid)

            xt8 = (xp.tile([P, 2, 2, T], FP8, tag="x8", name="x8")
                   if any8 else None)
            xtb = (xp.tile([P, KD, T], BF16, tag="xb", name="xb")
                   if anyb else None)
            wt8 = (wp.tile([P, len(g8), 2, 2, H], FP8, tag="w8", name="w8")
                   if any8 else None)
            wtb = (wp.tile([P, len(gb) * KD, H], BF16, tag="wb", name="wb")
                   if anyb else None)

            def load_x(q):
                tsl = slice(q * TQ, (q + 1) * TQ)
                if any8:
                    nc.sync.dma_start(xt8[:, :, :, tsl],
                                      x8_d.ap()[:, :, :, tsl])
                if anyb:
                    nc.sync.dma_start(xtb[:, :, tsl],
                                      xb_d.ap()[:, :, tsl])

            # DMA in fine-grained consumption order so the pipeline fills
            # fast: f-gate fp8 weights + first 512 x columns first (first
            # matmul can start ~2.5us in), then i-gate weights, the rest of
            # quarter 0, the h-gate weights (first needed lag_a2+lag_b units
            # in), and finally quarters 1..3.
            if any8:
                nc.sync.dma_start(wt8[:, 0], w8_d.ap()[:, 0])
                nc.sync.dma_start(xt8[:, :, :, 0:512],
                                  x8_d.ap()[:, :, :, 0:512])
                load_consts()
                nc.sync.dma_start(wt8[:, 1], w8_d.ap()[:, 1])
                nc.sync.dma_start(xt8[:, :, :, 512:TQ],
                                  x8_d.ap()[:, :, :, 512:TQ])
            if anyb:
                if not any8:
                    load_consts()
                nc.sync.dma_start(wtb[:], wb_d.ap())
                nc.sync.dma_start(xtb[:, :, 0:TQ], xb_d.ap()[:, :, 0:TQ])
            for q in range(1, NQ):
                load_x(q)

            def mm_group(z, g, hb, c0, width):
                """z[128, width] (PSUM) = W_g^T[., hb] @ x[., c0:c0+width]."""
                hsl = slice(hb * P, (hb + 1) * P)
                fp8 = gdt[g] == FP8
                nj = 2 if fp8 else KD
                if "mm1" in ablate:
                    nj = 1
                for j in range(nj):
                    for t2 in range(width // 512):
                        c = c0 + t2 * 512
                        zsl = z[:, t2 * 512:(t2 + 1) * 512]
                        if fp8:
                            nc.tensor.matmul(
                                zsl, wt8[:, g8.index(g), j, :, hsl],
                                xt8[:, j, :, c:c + 512],
                                start=(j == 0), stop=(j == nj - 1),
                                perf_mode=mybir.MatmulPerfMode.DoubleRow)
                        else:
                            nc.tensor.matmul(
                                zsl, wtb[:, gb.index(g) * KD + j, hsl],
                                xtb[:, j, c:c + 512],
                                start=(j == 0), stop=(j == nj - 1))

            def gbias(g, hb):
                return bias[:, g * HB + hb:g * HB + hb + 1]

            def stage_a(c0, w, hb):
                """zf/zi matmuls -> sigmoids (bf16 SBUF)."""
                tf = gp.tile([P, w], BF16, tag="tf")
                ti = gp.tile([P, w], BF16, tag="ti")
                for g, gt in ((0, tf), (1, ti)):
                    z = ps.tile([P, w], F32, tag="z")
                    if "mm" not in ablate:
                        mm_group(z, g, hb, c0, w)
                    nc.scalar.activation(gt[:], z[:], AF.Sigmoid,
                                         bias=gbias(g, hb))
                return (c0, w, hb, tf, ti)

            def stage_a2(st):
                """ts (PE idmm) -> rr (DVE recip) -> qq (Pool) -> aa (DVE)."""
                c0, w, hb, tf, ti = st
                if "nodiv" in ablate:
                    return (c0, w, hb, ti, ti)
                if ts_pe:
                    ts = tsp.tile([P, w], F32, tag="ts")
                    for t2 in range(w // 512):
                        sl = slice(t2 * 512, (t2 + 1) * 512)
                        nc.tensor.matmul(ts[:, sl], ident[:], tf[:, sl],
                                         start=True, stop=False)
                        nc.tensor.matmul(ts[:, sl], ident[:], ti[:, sl],
                                         start=False, stop=True)
                else:
                    ts = ep.tile([P, w], F32, tag="ts")
                    nc.vector.tensor_tensor(ts[:], tf[:], ti[:], ALU.add)
                rr = ep.tile([P, w], F32, tag="rr")
                nc.vector.reciprocal_approx_fast(rr[:], ts[:])
                qq = ep.tile([P, w], BF16, tag="qq")
                getattr(nc, qq_eng).tensor_tensor(qq[:], rr[:], ti[:],
                                                  ALU.mult)
                aa = ep.tile([P, w], BF16, tag="aa")
                nc.vector.tensor_scalar(aa[:], qq[:], -1.0, 1.0,
                                        ALU.mult, ALU.add)
                return (c0, w, hb, qq, aa)

            def stage_b1(st, k):
                """zh matmul -> th sigmoid (+ zp identity for gg_act units).

                Keeping the PSUM z tile alive for stage_b2's stt would pin a
                PSUM buffer across an extra pipeline stage, so gg_act units
                materialize zp in SBUF here; the others defer the z+bias add
                to an stt in b2 -- but that stt needs z, so non-gg_act units
                ALSO run their gg stt here, right after th.
                """
                c0, w, hb, qq, aa = st
                z = ps.tile([P, w], F32, tag="z")
                if "mm" not in ablate:
                    mm_group(z, 2, hb, c0, w)
                th = gp.tile([P, w], BF16, tag="th")
                nc.scalar.activation(th[:], z[:], AF.Sigmoid,
                                     bias=gbias(2, hb))
                gg = gp.tile([P, w], BF16, tag="gg")
                if k in gg_act_set:
                    # ACT materializes zp = zh + bh + 0.5; DVE TT max runs in
                    # the 2x bf16 mode (vs 1x for the stt fallback).
                    zp = gp.tile([P, w], BF16, tag="zp")
                    nc.scalar.activation(
                        zp[:], z[:], AF.Identity,
                        bias=bias[:, 3 * HB + hb:3 * HB + hb + 1])
                    nc.vector.tensor_tensor(gg[:], zp[:], th[:], ALU.max)
                else:
                    nc.vector.scalar_tensor_tensor(
                        gg[:], z[:], bias[:, 3 * HB + hb:3 * HB + hb + 1],
                        th[:], ALU.add, ALU.max)
                return (c0, w, hb, qq, aa, gg)

            def stage_b2(st, prev, k):
                """bb -> scan -> out (all inputs >= 1 iteration old)."""
                c0, w, hb, qq, aa, gg = st
                if "nodiv" in ablate:
                    bb = gg
                else:
                    bb = ep.tile([P, w], BF16, tag="bb")
                    beng = nc.gpsimd if k in bb_pool_set else nc.vector
                    beng.tensor_tensor(bb[:], qq[:], gg[:], ALU.mult)

                if "noscan" in ablate:
                    ho = bb
                else:
                    ho = hop.tile([P, w], out_dt, tag=f"ho{hb}")
                    if prev[hb] is None:
                        init = h0[:, hb:hb + 1]
                    else:
                        pho, pw = prev[hb]
                        init = pho[:, pw - 1:pw]
                    nc.vector.tensor_tensor_scan(ho[:], aa[:], bb[:], init,
                                                 ALU.mult, ALU.add)
                    prev[hb] = (ho, w)
                if "nodma" not in ablate:
                    nc.sync.dma_start(
                        ht_d.ap()[hb * P:(hb + 1) * P, c0:c0 + w], ho[:])

            import contextlib
            loop_cm = (tc.For_i(0, loop_reps, 1) if loop_reps
                       else contextlib.nullcontext())
            with loop_cm:
                # Flat unit list; final units split into 512-wide sub-chunks
                # to shorten the dependent tail after the last matmul.
                units = []
                for q in range(NQ):
                    for hb in range(HB):
                        subs = (2 if q * HB + hb >= NQ * HB - tail_units
                                else 1)
                        w = TQ // subs
                        for s in range(subs):
                            units.append((q * TQ + s * w, w, hb))
                n = len(units)
                # 4-stage pipeline, emission order per iteration:
                #   b1(k-2): zh matmul + th/zp/gg -- PE/ACT/DVE heads get
                #            work whose inputs are already computed;
                #   b2(k-3): bb/scan/out -- every input >= 1 iteration old,
                #            so the DVE head never waits;
                #   a2(k-1): ts idmm right after zh on PE (sigma(k-1) is an
                #            iteration old), rr/aa at the DVE tail;
                #   a(k):    zf/zi matmuls + sigmoids last.
                prev = [None] * HB
                st_a = {}
                st_a2 = {}
                st_b1 = {}
                b2_lag = 0 if b2_same else 1
                for it in range(n + lag_a2 + lag_b + b2_lag):
                    if not b2_same:
                        kb2 = it - lag_a2 - lag_b - 1
                        if kb2 >= 0:
                            stage_b2(st_b1.pop(kb2), prev, kb2)
                    kb1 = it - lag_a2 - lag_b
                    if 0 <= kb1 < n:
                        st_b1[kb1] = stage_b1(st_a2.pop(kb1), kb1)
                        if b2_same:
                            stage_b2(st_b1.pop(kb1), prev, kb1)
                    if it < n:
                        st_a[it] = stage_a(*units[it])
                    ka = it - lag_a2
                    if 0 <= ka < n:
                        st_a2[ka] = stage_a2(st_a.pop(ka))

    nc.compile()
    return nc


def _get_module():
    if "nc" not in _CACHE:
        _CACHE["nc"] = _build()
    return _CACHE["nc"]


class _Runner:
    """Caches a compiled 8-core shard_map'd PJRT executable of the Bass
    module so repeat kernel() calls skip jax retracing/compilation."""

    def __init__(self, nc):
        import jax
        from jax.experimental.shard_map import shard_map
        from jax.sharding import Mesh, PartitionSpec

        from concourse import bass2jax

        bass2jax.install_neuronx_cc_hook()
        self.nc = nc
        partition_name = (nc.partition_id_tensor.name
                          if nc.partition_id_tensor else None)
        in_names, out_names, out_avals = [], [], []
        for alloc in nc.m.functions[0].allocations:
            if not isinstance(alloc, mybir.MemoryLocationSet):
                continue
            name = alloc.memorylocations[0].name
            if alloc.kind == "ExternalInput":
                if name != partition_name:
                    in_names.append(name)
            elif alloc.kind == "ExternalOutput":
                out_names.append(name)
                out_avals.append(jax.core.ShapedArray(
                    tuple(alloc.tensor_shape), mybir.dt.np(alloc.dtype)))
        self.in_names = in_names
        self.out_names = out_names
        self.out_avals = out_avals
        n_params, n_outs = len(in_names), len(out_names)
        all_names = list(in_names) + list(out_names)
        if partition_name is not None:
            all_names.append(partition_name)

        def _body(*args):
            operands = list(args)
            if partition_name is not None:
                operands.append(bass2jax.partition_id_tensor())
            return tuple(bass2jax._bass_exec_p.bind(
                *operands,
                out_avals=tuple(out_avals),
                in_names=tuple(all_names),
                out_names=tuple(out_names),
                lowering_input_output_aliases=(),
                sim_require_finite=True,
                sim_require_nnan=True,
                nc=nc,
            ))

        devices = jax.devices()[:B]
        mesh = Mesh(np.asarray(devices), ("core",))
        specs = (PartitionSpec("core"),) * (n_params + n_outs)
        out_specs = (PartitionSpec("core"),) * n_outs
        donate = tuple(range(n_params, n_params + n_outs))
        self._jitted = jax.jit(
            shard_map(_body, mesh=mesh, in_specs=specs,
                      out_specs=out_specs, check_rep=False),
            donate_argnums=donate, keep_unused=True)
        self._compiled = None

    def concat_args(self, in_maps):
        concat_in = [
            np.concatenate([np.asarray(m[name]) for m in in_maps], axis=0)
            for name in self.in_names
        ]
        concat_zeros = [
            np.zeros((B * a.shape[0], *a.shape[1:]), a.dtype)
            for a in self.out_avals
        ]
        return concat_in + concat_zeros

    def compiled(self, args):
        if self._compiled is None:
            self._compiled = self._jitted.lower(*args).compile()
        return self._compiled

    def __call__(self, in_maps):
        import jax
        args = self.concat_args(in_maps)
        outs = jax.block_until_ready(self.compiled(args)(*args))
        return [
            {name: np.asarray(outs[i]).reshape(B, *self.out_avals[i].shape)[c]
             for i, name in enumerate(self.out_names)}
            for c in range(B)
        ]


def _get_runner():
    if "runner" not in _CACHE:
        _CACHE["runner"] = _Runner(_get_module())
    return _CACHE["runner"]


def make_in_maps(x, h_0, W_f, b_f, W_i, b_i, W_h, b_h, mm=MM, ts_pe=True):
    gdt = _gate_dtypes(mm)
    any8 = any(d == FP8 for d in gdt.values())
    anyb = any(d == BF16 for d in gdt.values())
    f8 = mybir.dt.np(FP8)
    bf = mybir.dt.np(BF16)
    x = np.asarray(x, np.float32)
    h_0 = np.asarray(h_0, np.float32)
    wT = np.ascontiguousarray(
        np.stack([np.asarray(W_f), np.asarray(W_i), np.asarray(W_h)])
        .astype(np.float32).transpose(0, 2, 1))          # [3, D, H]
    b_h = np.asarray(b_h)
    biasp = np.ascontiguousarray(
        np.stack([np.asarray(b_f), np.asarray(b_i), b_h, b_h + 0.5])
        .astype(np.float32).reshape(4, HB, P).transpose(2, 0, 1)
        .reshape(P, 4 * HB))
    g8 = [g for g in range(3) if gdt[g] == FP8]
    gb = [g for g in range(3) if gdt[g] == BF16]
    common = {"biasp": biasp}
    if ts_pe:
        common["identp"] = np.eye(P, dtype=bf)
    if any8:
        # [3, D, H] -> [p, gi, j, i, H] with d = (2j+i)*128+p
        w8 = wT[g8].reshape(len(g8), 2, 2, P, H)         # [gi, j, i, p, H]
        common["wT8"] = np.ascontiguousarray(
            w8.transpose(3, 0, 1, 2, 4)).astype(f8)
    if anyb:
        # [3, D, H] -> [p, ki, H] with d = k*128+p, ki = gb_idx*KD + k
        wb = wT[gb].reshape(len(gb), KD, P, H)           # [gi, k, p, H]
        common["wTb"] = np.ascontiguousarray(
            wb.transpose(2, 0, 1, 3).reshape(P, len(gb) * KD, H)).astype(bf)
    in_maps = []
    for b in range(B):
        xT = np.ascontiguousarray(x[b].T)                # [D, T]
        m = dict(common)
        if any8:
            # [D, T] -> [p, j, i, T]
            m["xT8"] = np.ascontiguousarray(
                xT.reshape(2, 2, P, T).transpose(2, 0, 1, 3)).astype(f8)
        if anyb:
            # [D, T] -> [p, k, T]
            m["xTb"] = np.ascontiguousarray(
                xT.reshape(KD, P, T).transpose(1, 0, 2)).astype(bf)
        m["h0p"] = np.ascontiguousarray(h_0[b].reshape(HB, P).T)
        in_maps.append(m)
    return in_maps


def kernel(x, h_0, W_f, b_f, W_i, b_i, W_h, b_h):
    in_maps = make_in_maps(x, h_0, W_f, b_f, W_i, b_i, W_h, b_h)
    results = _get_runner()(in_maps)
    out = np.empty((B, T, H), np.float32)
    for b in range(B):
        out[b] = results[b]["ht"].astype(np.float32).T
    return out


# revision 26
# speedup vs baseline: 1.1518x; 1.1518x over previous
"""MinLSTM Trainium2 kernel (v2: engine-rebalanced).

Math (identical to the log-space reference, in linear space):
    sf = sigmoid(x @ W_f.T + b_f)
    si = sigmoid(x @ W_i.T + b_i)
    zh = x @ W_h.T + b_h
    g  = max(zh + 0.5, sigmoid(zh))           (exact rewrite of log_g)
    aa = sf / (sf + si)                        (normalized forget gate)
    bb = si * g / (sf + si)                    (normalized input contribution)
    h_t = aa_t * h_{t-1} + bb_t                (hardware tensor_tensor_scan)

Sharding: data-parallel over batch B=8, one batch per NeuronCore. Host
pre-transposes x[b] to [D, T] so gate matmuls produce z in [H-partition,
T-free] layout, which the per-partition scan along the free dim needs.

v2 engine assignment (v1 was DVE-bound at ~84% busy; HW-ablated 2026-08):
  - ts = sf+si moved from DVE TensorTensor to the PE: two identity-weight
    bf16 matmuls accumulating into a PSUM tile (PE had ~45% headroom).
    HW: removing this costs +20us (141 vs 121).
  - aa = 1-qq moved from ACT Identity to DVE tensor_scalar (TSP fast
    modes). HW: -5.6us vs the ACT variant.
  - qq = rr*si stays on Pool (GpSimd TT mult, its only fast op).
  - gg = max(zh+bh+.5, sigmoid(zh+bh)): mostly one DVE stt; gg_act=2
    units use ACT Identity zp + DVE TT max to shave DVE load.
  - bb = qq*gg on DVE TT. Pool-bb looked fine in the cost model but sits
    on the scan critical path behind the 2.1us Pool mult: bb_pool=8
    measured +27us on HW. Keep bb_pool=0.
  - scans are DVE tensor_tensor_scan (DVE-only op, no fast mode).

Three-stage software pipeline, emission order per iteration
  b(k-2) [zh mm + th/gg + bb/scan/out], a(k) [f/i mms + sigmas],
  a2(k-1) [ts idmm, rr, qq, aa]
so every engine's queue head only carries satisfied or nearly-satisfied
waits (in-order engine queues head-of-line block otherwise), and the
startup DMAs are ordered bias -> f-weights -> first x chunks so the
first sigmoid fires ~5us in (the ACT sigmoid table is pre-loaded from a
MemsetZero'd tile, not a DMA'd one).

mm="hybrid": f,i gate matmuls in fp8e4 with DoubleRow (2x PE rate,
quarter DMA), h gate in bf16 (g is linear in zh for zh>0, so fp8 x
quantization there would not cancel).
"""

import os
import sys

for _p in ("/opt/trn_rl_repo", "/root/.axon_site/_ro/trn_rl_repo"):
    if os.path.isdir(_p) and _p not in sys.path:
        sys.path.insert(0, _p)

import numpy as np

import concourse.bacc as bacc
import concourse.tile as tile
from concourse import bass_utils, mybir
from concourse.mybir import ActivationFunctionType as AF
from concourse.mybir import AluOpType as ALU

B, T, D, H = 8, 4096, 512, 512
P = 128
KD = D // P       # 4 contraction blocks
HB = H // P       # 4 hidden-partition blocks
TQ = 1024         # matmul/unit width (2 fp32 PSUM banks)
NQ = T // TQ      # 4 quarters
F32 = mybir.dt.float32
BF16 = mybir.dt.bfloat16
FP8 = mybir.dt.float8e4

MM = "hybrid"     # "bf16" | "fp8" | "hybrid" (f,i gates fp8; h gate bf16)
OUT_DT = BF16

_CACHE = {}


def _gate_dtypes(mm):
    """Per-gate matmul dtype: gates (0=f, 1=i, 2=h)."""
    if mm == "bf16":
        return {0: BF16, 1: BF16, 2: BF16}
    if mm == "fp8":
        return {0: FP8, 1: FP8, 2: FP8}
    if mm == "hybrid":
        return {0: FP8, 1: FP8, 2: BF16}
    raise ValueError(mm)


def _spread(n, total=16):
    """Bresenham-spread set of n unit indices out of `total`."""
    if n <= 0:
        return set()
    return {(i * total) // n for i in range(n)}


def _build(n_cores=B, loop_reps=0, mm=MM, out_dt=OUT_DT, ablate=(),
           gg_act=2, bb_pool=0, qq_eng="gpsimd", ts_pe=True, aa_eng="vector",
           lag_a2=1, lag_b=1, bufs=5, hop_bufs=2, tail_units=2,
           head_units=0, z_bufs=3, ts_bufs=1, b2_same=True):
    gdt = _gate_dtypes(mm)
    any8 = any(d == FP8 for d in gdt.values())
    anyb = any(d == BF16 for d in gdt.values())
    nc = bacc.Bacc("TRN2", target_bir_lowering=False, debug=False,
                   num_devices=n_cores)
    g8 = [g for g in range(3) if gdt[g] == FP8]   # fp8 gates
    gb = [g for g in range(3) if gdt[g] == BF16]  # bf16 gates
    # All tensors partition-major so each load is ONE dma_start:
    #   x8 [p, j, i, t]   with d = (2j+i)*128+p (DoubleRow pairs)
    #   xb [p, k, t]      with d = k*128+p
    #   w8 [p, gi, j, i, h] / wb [p, ki, h]  (gi/ki index into g8/gb lists)
    x8_d = (nc.dram_tensor("xT8", [P, 2, 2, T], FP8, kind="ExternalInput")
            if any8 else None)
    xb_d = (nc.dram_tensor("xTb", [P, KD, T], BF16, kind="ExternalInput")
            if anyb else None)
    w8_d = (nc.dram_tensor("wT8", [P, len(g8), 2, 2, H], FP8,
                           kind="ExternalInput") if any8 else None)
    wb_d = (nc.dram_tensor("wTb", [P, len(gb) * KD, H], BF16,
                           kind="ExternalInput") if anyb else None)
    # 4 bias groups packed per partition: [b_f | b_i | b_h | b_h + 0.5]
    bias_d = nc.dram_tensor("biasp", [P, 4 * HB], F32, kind="ExternalInput")
    h0_d = nc.dram_tensor("h0p", [P, HB], F32, kind="ExternalInput")
    id_d = (nc.dram_tensor("identp", [P, P], BF16, kind="ExternalInput")
            if ts_pe else None)
    ht_d = nc.dram_tensor("ht", [H, T], out_dt, kind="ExternalOutput")

    gg_act_set = _spread(gg_act)
    bb_pool_set = _spread(bb_pool)

    with tile.TileContext(nc) as tc:
        with (
            tc.tile_pool(name="xp", bufs=1) as xp,
            tc.tile_pool(name="wp", bufs=1) as wp,
            tc.tile_pool(name="cp", bufs=1) as cp,
            tc.tile_pool(name="ps", bufs=z_bufs, space="PSUM") as ps,
            tc.tile_pool(name="tsp", bufs=ts_bufs, space="PSUM") as tsp,
            tc.tile_pool(name="gp", bufs=bufs) as gp,
            tc.tile_pool(name="ep", bufs=bufs) as ep,
            tc.tile_pool(name="hop", bufs=hop_bufs) as hop,
        ):
            bias = cp.tile([P, 4 * HB], F32, tag="bias")
            h0 = cp.tile([P, HB], F32, tag="h0")
            ident = (cp.tile([P, P], BF16, tag="ident", name="ident")
                     if ts_pe else None)
            warm = cp.tile([P, 1], F32, tag="warm")

            # The bias DMA goes first (tiny, and it gates the first
            # sigmoid); the warm-up sigmoid off it pre-loads the ACT
            # sigmoid table (~1.3us) before the first real sigmoid.
            nc.sync.dma_start(bias[:], bias_d.ap())
            nc.scalar.activation(warm[:], bias[:, 0:1], AF.Sigmoid)

            def load_consts():
                """Emitted after the first matmul-feeding DMAs: ident is
                first needed by the first idmm (~6us), h0 by the first scan
                (~10us) -- neither on the startup critical path."""
                if ts_pe:
                    nc.sync.dma_start(ident[:], id_d.ap())
                nc.sync.dma_start(h0[:], h0_d.ap())

            xt8 = (xp.tile([P, 2, 2, T], FP8, tag="x8", name="x8")
                   if any8 else None)
            xtb = (xp.tile([P, KD, T], BF16, tag="xb", name="xb")
                   if anyb else None)
            wt8 = (wp.tile([P, len(g8), 2, 2, H], FP8, tag="w8", name="w8")
                   if any8 else None)
            wtb = (wp.tile([P, len(gb) * KD, H], BF16, tag="wb", name="wb")
                   if anyb else None)

            def load_x(q):
                tsl = slice(q * TQ, (q + 1) * TQ)
                if any8:
                    nc.sync.dma_start(xt8[:, :, :, tsl],
                                      x8_d.ap()[:, :, :, tsl])
                if anyb:
                    nc.sync.dma_start(xtb[:, :, tsl],
                                      xb_d.ap()[:, :, tsl])

            # DMA in fine-grained consumption order so the pipeline fills
            # fast: f-gate fp8 weights + first 512 x columns first (first
            # matmul can start ~2.5us in), then i-gate weights, the rest of
            # quarter 0, the h-gate weights (first needed lag_a2+lag_b units
            # in), and finally quarters 1..3.
            if any8:
                nc.sync.dma_start(wt8[:, 0], w8_d.ap()[:, 0])
                nc.sync.dma_start(xt8[:, :, :, 0:512],
                                  x8_d.ap()[:, :, :, 0:512])
                nc.sync.dma_start(wt8[:, 1], w8_d.ap()[:, 1])
                nc.sync.dma_start(xt8[:, :, :, 512:TQ],
                                  x8_d.ap()[:, :, :, 512:TQ])
                load_consts()
            if anyb:
                if not any8:
                    load_consts()
                nc.sync.dma_start(wtb[:], wb_d.ap())
                nc.sync.dma_start(xtb[:, :, 0:TQ], xb_d.ap()[:, :, 0:TQ])
            for q in range(1, NQ):
                load_x(q)

            def mm_group(z, g, hb, c0, width):
                """z[128, width] (PSUM) = W_g^T[., hb] @ x[., c0:c0+width]."""
                hsl = slice(hb * P, (hb + 1) * P)
                fp8 = gdt[g] == FP8
                nj = 2 if fp8 else KD
                if "mm1" in ablate:
                    nj = 1
                for j in range(nj):
                    for t2 in range(width // 512):
                        c = c0 + t2 * 512
                        zsl = z[:, t2 * 512:(t2 + 1) * 512]
                        if fp8:
                            nc.tensor.matmul(
                                zsl, wt8[:, g8.index(g), j, :, hsl],
                                xt8[:, j, :, c:c + 512],
                                start=(j == 0), stop=(j == nj - 1),
                                perf_mode=mybir.MatmulPerfMode.DoubleRow)
                        else:
                            nc.tensor.matmul(
                                zsl, wtb[:, gb.index(g) * KD + j, hsl],
                                xtb[:, j, c:c + 512],
                                start=(j == 0), stop=(j == nj - 1))

            def gbias(g, hb):
                return bias[:, g * HB + hb:g * HB + hb + 1]

            def stage_a(c0, w, hb):
                """zf/zi matmuls -> sigmoids (bf16 SBUF)."""
                tf = gp.tile([P, w], BF16, tag="tf")
                ti = gp.tile([P, w], BF16, tag="ti")
                for g, gt in ((0, tf), (1, ti)):
                    z = ps.tile([P, w], F32, tag="z")
                    if "mm" not in ablate:
                        mm_group(z, g, hb, c0, w)
                    nc.scalar.activation(gt[:], z[:], AF.Sigmoid,
                                         bias=gbias(g, hb))
                return (c0, w, hb, tf, ti)

            def stage_a2(st):
                """ts (PE idmm) -> rr (DVE recip) -> qq (Pool) -> aa (DVE)."""
                c0, w, hb, tf, ti = st
                if "nodiv" in ablate:
                    return (c0, w, hb, ti, ti)
                if ts_pe:
                    ts = tsp.tile([P, w], F32, tag="ts")
                    for t2 in range(w // 512):
                        sl = slice(t2 * 512, (t2 + 1) * 512)
                        nc.tensor.matmul(ts[:, sl], ident[:], tf[:, sl],
                                         start=True, stop=False)
                        nc.tensor.matmul(ts[:, sl], ident[:], ti[:, sl],
                                         start=False, stop=True)
                else:
                    ts = ep.tile([P, w], F32, tag="ts")
                    nc.vector.tensor_tensor(ts[:], tf[:], ti[:], ALU.add)
                rr = ep.tile([P, w], F32, tag="rr")
                nc.vector.reciprocal_approx_fast(rr[:], ts[:])
                qq = ep.tile([P, w], BF16, tag="qq")
                getattr(nc, qq_eng).tensor_tensor(qq[:], rr[:], ti[:],
                                                  ALU.mult)
                aa = ep.tile([P, w], BF16, tag="aa")
                if aa_eng == "scalar":
                    nc.scalar.activation(aa[:], qq[:], AF.Identity,
                                         bias=1.0, scale=-1.0)
                else:
                    nc.vector.tensor_scalar(aa[:], qq[:], -1.0, 1.0,
                                            ALU.mult, ALU.add)
                return (c0, w, hb, qq, aa)

            def stage_b1(st, k):
                """zh matmul -> th sigmoid (+ zp identity for gg_act units).

                Keeping the PSUM z tile alive for stage_b2's stt would pin a
                PSUM buffer across an extra pipeline stage, so gg_act units
                materialize zp in SBUF here; the others defer the z+bias add
                to an stt in b2 -- but that stt needs z, so non-gg_act units
                ALSO run their gg stt here, right after th.
                """
                c0, w, hb, qq, aa = st
                z = ps.tile([P, w], F32, tag="z")
                if "mm" not in ablate:
                    mm_group(z, 2, hb, c0, w)
                th = gp.tile([P, w], BF16, tag="th")
                nc.scalar.activation(th[:], z[:], AF.Sigmoid,
                                     bias=gbias(2, hb))
                gg = gp.tile([P, w], BF16, tag="gg")
                if k in gg_act_set:
                    # ACT materializes zp = zh + bh + 0.5; DVE TT max runs in
                    # the 2x bf16 mode (vs 1x for the stt fallback).
                    zp = gp.tile([P, w], BF16, tag="zp")
                    nc.scalar.activation(
                        zp[:], z[:], AF.Identity,
                        bias=bias[:, 3 * HB + hb:3 * HB + hb + 1])
                    nc.vector.tensor_tensor(gg[:], zp[:], th[:], ALU.max)
                else:
                    nc.vector.scalar_tensor_tensor(
                        gg[:], z[:], bias[:, 3 * HB + hb:3 * HB + hb + 1],
                        th[:], ALU.add, ALU.max)
                return (c0, w, hb, qq, aa, gg)

            def stage_b2(st, prev, k):
                """bb -> scan -> out (all inputs >= 1 iteration old)."""
                c0, w, hb, qq, aa, gg = st
                if "nodiv" in ablate:
                    bb = gg
                else:
                    bb = ep.tile([P, w], BF16, tag="bb")
                    beng = nc.gpsimd if k in bb_pool_set else nc.vector
                    beng.tensor_tensor(bb[:], qq[:], gg[:], ALU.mult)

                if "noscan" in ablate:
                    ho = bb
                else:
                    ho = hop.tile([P, w], out_dt, tag=f"ho{hb}")
                    if prev[hb] is None:
                        init = h0[:, hb:hb + 1]
                    else:
                        pho, pw = prev[hb]
                        init = pho[:, pw - 1:pw]
                    nc.vector.tensor_tensor_scan(ho[:], aa[:], bb[:], init,
                                                 ALU.mult, ALU.add)
                    prev[hb] = (ho, w)
                if "nodma" not in ablate:
                    nc.sync.dma_start(
                        ht_d.ap()[hb * P:(hb + 1) * P, c0:c0 + w], ho[:])

            import contextlib
            loop_cm = (tc.For_i(0, loop_reps, 1) if loop_reps
                       else contextlib.nullcontext())
            with loop_cm:
                # Flat unit list; final units split into 512-wide sub-chunks
                # to shorten the dependent tail after the last matmul.
                units = []
                for q in range(NQ):
                    for hb in range(HB):
                        u = q * HB + hb
                        # split head units (pipeline fills at half-unit
                        # granularity while DMAs stream in) and tail units
                        # (shortens the dependent tail after the last mm)
                        subs = (2 if (u >= NQ * HB - tail_units
                                      or u < head_units) else 1)
                        w = TQ // subs
                        for s in range(subs):
                            units.append((q * TQ + s * w, w, hb))
                n = len(units)
                # 4-stage pipeline, emission order per iteration:
                #   b1(k-2): zh matmul + th/zp/gg -- PE/ACT/DVE heads get
                #            work whose inputs are already computed;
                #   b2(k-3): bb/scan/out -- every input >= 1 iteration old,
                #            so the DVE head never waits;
                #   a2(k-1): ts idmm right after zh on PE (sigma(k-1) is an
                #            iteration old), rr/aa at the DVE tail;
                #   a(k):    zf/zi matmuls + sigmoids last.
                prev = [None] * HB
                st_a = {}
                st_a2 = {}
                st_b1 = {}
                b2_lag = 0 if b2_same else 1
                for it in range(n + lag_a2 + lag_b + b2_lag):
                    if not b2_same:
                        kb2 = it - lag_a2 - lag_b - 1
                        if kb2 >= 0:
                            stage_b2(st_b1.pop(kb2), prev, kb2)
                    kb1 = it - lag_a2 - lag_b
                    if 0 <= kb1 < n:
                        st_b1[kb1] = stage_b1(st_a2.pop(kb1), kb1)
                        if b2_same:
                            stage_b2(st_b1.pop(kb1), prev, kb1)
                    if it < n:
                        st_a[it] = stage_a(*units[it])
                    ka = it - lag_a2
                    if 0 <= ka < n:
                        st_a2[ka] = stage_a2(st_a.pop(ka))

    nc.compile()
    return nc


def _get_module():
    if "nc" not in _CACHE:
        _CACHE["nc"] = _build()
    return _CACHE["nc"]


class _Runner:
    """Caches a compiled 8-core shard_map'd PJRT executable of the Bass
    module so repeat kernel() calls skip jax retracing/compilation."""

    def __init__(self, nc):
        import jax
        from jax.experimental.shard_map import shard_map
        from jax.sharding import Mesh, PartitionSpec

        from concourse import bass2jax

        bass2jax.install_neuronx_cc_hook()
        self.nc = nc
        partition_name = (nc.partition_id_tensor.name
                          if nc.partition_id_tensor else None)
        in_names, out_names, out_avals = [], [], []
        for alloc in nc.m.functions[0].allocations:
            if not isinstance(alloc, mybir.MemoryLocationSet):
                continue
            name = alloc.memorylocations[0].name
            if alloc.kind == "ExternalInput":
                if name != partition_name:
                    in_names.append(name)
            elif alloc.kind == "ExternalOutput":
                out_names.append(name)
                out_avals.append(jax.core.ShapedArray(
                    tuple(alloc.tensor_shape), mybir.dt.np(alloc.dtype)))
        self.in_names = in_names
        self.out_names = out_names
        self.out_avals = out_avals
        n_params, n_outs = len(in_names), len(out_names)
        all_names = list(in_names) + list(out_names)
        if partition_name is not None:
            all_names.append(partition_name)

        def _body(*args):
            operands = list(args)
            if partition_name is not None:
                operands.append(bass2jax.partition_id_tensor())
            return tuple(bass2jax._bass_exec_p.bind(
                *operands,
                out_avals=tuple(out_avals),
                in_names=tuple(all_names),
                out_names=tuple(out_names),
                lowering_input_output_aliases=(),
                sim_require_finite=True,
                sim_require_nnan=True,
                nc=nc,
            ))

        devices = jax.devices()[:B]
        mesh = Mesh(np.asarray(devices), ("core",))
        specs = (PartitionSpec("core"),) * (n_params + n_outs)
        out_specs = (PartitionSpec("core"),) * n_outs
        donate = tuple(range(n_params, n_params + n_outs))
        self._jitted = jax.jit(
            shard_map(_body, mesh=mesh, in_specs=specs,
                      out_specs=out_specs, check_rep=False),
            donate_argnums=donate, keep_unused=True)
        self._compiled = None

    def concat_args(self, in_maps):
        concat_in = [
            np.concatenate([np.asarray(m[name]) for m in in_maps], axis=0)
            for name in self.in_names
        ]
        concat_zeros = [
            np.zeros((B * a.shape[0], *a.shape[1:]), a.dtype)
            for a in self.out_avals
        ]
        return concat_in + concat_zeros

    def compiled(self, args):
        if self._compiled is None:
            self._compiled = self._jitted.lower(*args).compile()
        return self._compiled

    def __call__(self, in_maps):
        import jax
        args = self.concat_args(in_maps)
        outs = jax.block_until_ready(self.compiled(args)(*args))
        return [
            {name: np.asarray(outs[i]).reshape(B, *self.out_avals[i].shape)[c]
             for i, name in enumerate(self.out_names)}
            for c in range(B)
        ]


def _get_runner():
    if "runner" not in _CACHE:
        _CACHE["runner"] = _Runner(_get_module())
    return _CACHE["runner"]


def make_in_maps(x, h_0, W_f, b_f, W_i, b_i, W_h, b_h, mm=MM, ts_pe=True):
    gdt = _gate_dtypes(mm)
    any8 = any(d == FP8 for d in gdt.values())
    anyb = any(d == BF16 for d in gdt.values())
    f8 = mybir.dt.np(FP8)
    bf = mybir.dt.np(BF16)
    x = np.asarray(x, np.float32)
    h_0 = np.asarray(h_0, np.float32)
    wT = np.ascontiguousarray(
        np.stack([np.asarray(W_f), np.asarray(W_i), np.asarray(W_h)])
        .astype(np.float32).transpose(0, 2, 1))          # [3, D, H]
    b_h = np.asarray(b_h)
    biasp = np.ascontiguousarray(
        np.stack([np.asarray(b_f), np.asarray(b_i), b_h, b_h + 0.5])
        .astype(np.float32).reshape(4, HB, P).transpose(2, 0, 1)
        .reshape(P, 4 * HB))
    g8 = [g for g in range(3) if gdt[g] == FP8]
    gb = [g for g in range(3) if gdt[g] == BF16]
    common = {"biasp": biasp}
    if ts_pe:
        common["identp"] = np.eye(P, dtype=bf)
    if any8:
        # [3, D, H] -> [p, gi, j, i, H] with d = (2j+i)*128+p
        w8 = wT[g8].reshape(len(g8), 2, 2, P, H)         # [gi, j, i, p, H]
        common["wT8"] = np.ascontiguousarray(
            w8.transpose(3, 0, 1, 2, 4)).astype(f8)
    if anyb:
        # [3, D, H] -> [p, ki, H] with d = k*128+p, ki = gb_idx*KD + k
        wb = wT[gb].reshape(len(gb), KD, P, H)           # [gi, k, p, H]
        common["wTb"] = np.ascontiguousarray(
            wb.transpose(2, 0, 1, 3).reshape(P, len(gb) * KD, H)).astype(bf)
    in_maps = []
    for b in range(B):
        xT = np.ascontiguousarray(x[b].T)                # [D, T]
        m = dict(common)
        if any8:
            # [D, T] -> [p, j, i, T]
            m["xT8"] = np.ascontiguousarray(
                xT.reshape(2, 2, P, T).transpose(2, 0, 1, 3)).astype(f8)
        if anyb:
            # [D, T] -> [p, k, T]
            m["xTb"] = np.ascontiguousarray(
                xT.reshape(KD, P, T).transpose(1, 0, 2)).astype(bf)
        m["h0p"] = np.ascontiguousarray(h_0[b].reshape(HB, P).T)
        in_maps.append(m)
    return in_maps


def kernel(x, h_0, W_f, b_f, W_i, b_i, W_h, b_h):
    in_maps = make_in_maps(x, h_0, W_f, b_f, W_i, b_i, W_h, b_h)
    results = _get_runner()(in_maps)
    out = np.empty((B, T, H), np.float32)
    for b in range(B):
        out[b] = results[b]["ht"].astype(np.float32).T
    return out


# revision 28
# speedup vs baseline: 1.1944x; 1.0370x over previous
"""MinLSTM Trainium2 kernel (v2: engine-rebalanced).

Math (identical to the log-space reference, in linear space):
    sf = sigmoid(x @ W_f.T + b_f)
    si = sigmoid(x @ W_i.T + b_i)
    zh = x @ W_h.T + b_h
    g  = max(zh + 0.5, sigmoid(zh))           (exact rewrite of log_g)
    aa = sf / (sf + si)                        (normalized forget gate)
    bb = si * g / (sf + si)                    (normalized input contribution)
    h_t = aa_t * h_{t-1} + bb_t                (hardware tensor_tensor_scan)

Sharding: data-parallel over batch B=8, one batch per NeuronCore. Host
pre-transposes x[b] to [D, T] so gate matmuls produce z in [H-partition,
T-free] layout, which the per-partition scan along the free dim needs.

v2 engine assignment (v1 was DVE-bound at ~84% busy; HW-ablated 2026-08):
  - ts = sf+si moved from DVE TensorTensor to the PE: two identity-weight
    bf16 matmuls accumulating into a PSUM tile (PE had ~45% headroom).
    HW: removing this costs +20us (141 vs 121).
  - aa = 1-qq moved from ACT Identity to DVE tensor_scalar (TSP fast
    modes). HW: -5.6us vs the ACT variant.
  - qq = rr*si stays on Pool (GpSimd TT mult, its only fast op).
  - gg = max(zh+bh+.5, sigmoid(zh+bh)): mostly one DVE stt; gg_act=2
    units use ACT Identity zp + DVE TT max to shave DVE load.
  - bb = qq*gg on DVE TT. Pool-bb looked fine in the cost model but sits
    on the scan critical path behind the 2.1us Pool mult: bb_pool=8
    measured +27us on HW. Keep bb_pool=0.
  - scans are DVE tensor_tensor_scan (DVE-only op, no fast mode).

Three-stage software pipeline, emission order per iteration
  b(k-2) [zh mm + th/gg + bb/scan/out], a(k) [f/i mms + sigmas],
  a2(k-1) [ts idmm, rr, qq, aa]
so every engine's queue head only carries satisfied or nearly-satisfied
waits (in-order engine queues head-of-line block otherwise), and the
startup DMAs are ordered bias -> f-weights -> first x chunks so the
first sigmoid fires ~5us in (the ACT sigmoid table is pre-loaded from a
MemsetZero'd tile, not a DMA'd one).

mm="hybrid": f,i gate matmuls in fp8e4 with DoubleRow (2x PE rate,
quarter DMA), h gate in bf16 (g is linear in zh for zh>0, so fp8 x
quantization there would not cancel).
"""

import os
import sys

for _p in ("/opt/trn_rl_repo", "/root/.axon_site/_ro/trn_rl_repo"):
    if os.path.isdir(_p) and _p not in sys.path:
        sys.path.insert(0, _p)

import numpy as np

import concourse.bacc as bacc
import concourse.tile as tile
from concourse import bass_utils, mybir
from concourse.mybir import ActivationFunctionType as AF
from concourse.mybir import AluOpType as ALU

B, T, D, H = 8, 4096, 512, 512
P = 128
KD = D // P       # 4 contraction blocks
HB = H // P       # 4 hidden-partition blocks
TQ = 1024         # matmul/unit width (2 fp32 PSUM banks)
NQ = T // TQ      # 4 quarters
F32 = mybir.dt.float32
BF16 = mybir.dt.bfloat16
FP8 = mybir.dt.float8e4

MM = "hybrid"     # "bf16" | "fp8" | "hybrid" (f,i gates fp8; h gate bf16)
OUT_DT = BF16

_CACHE = {}


def _gate_dtypes(mm):
    """Per-gate matmul dtype: gates (0=f, 1=i, 2=h)."""
    if mm == "bf16":
        return {0: BF16, 1: BF16, 2: BF16}
    if mm == "fp8":
        return {0: FP8, 1: FP8, 2: FP8}
    if mm == "hybrid":
        return {0: FP8, 1: FP8, 2: BF16}
    raise ValueError(mm)


def _spread(n, total=16):
    """Bresenham-spread set of n unit indices out of `total`."""
    if n <= 0:
        return set()
    return {(i * total) // n for i in range(n)}


def _build(n_cores=B, loop_reps=0, mm=MM, out_dt=OUT_DT, ablate=(),
           gg_act=2, bb_pool=0, qq_eng="gpsimd", ts_pe=True, aa_eng="vector",
           lag_a2=1, lag_b=1, bufs=5, hop_bufs=2, tail_units=2,
           head_units=0, z_bufs=3, ts_bufs=1, b2_same=True):
    gdt = _gate_dtypes(mm)
    any8 = any(d == FP8 for d in gdt.values())
    anyb = any(d == BF16 for d in gdt.values())
    nc = bacc.Bacc("TRN2", target_bir_lowering=False, debug=False,
                   num_devices=n_cores)
    g8 = [g for g in range(3) if gdt[g] == FP8]   # fp8 gates
    gb = [g for g in range(3) if gdt[g] == BF16]  # bf16 gates
    # All tensors partition-major so each load is ONE dma_start:
    #   x8 [p, j, i, t]   with d = (2j+i)*128+p (DoubleRow pairs)
    #   xb [p, k, t]      with d = k*128+p
    #   w8 [p, gi, j, i, h] / wb [p, ki, h]  (gi/ki index into g8/gb lists)
    x8_d = (nc.dram_tensor("xT8", [P, 2, 2, T], FP8, kind="ExternalInput")
            if any8 else None)
    xb_d = (nc.dram_tensor("xTb", [P, KD, T], BF16, kind="ExternalInput")
            if anyb else None)
    w8_d = (nc.dram_tensor("wT8", [P, len(g8), 2, 2, H], FP8,
                           kind="ExternalInput") if any8 else None)
    wb_d = (nc.dram_tensor("wTb", [P, len(gb) * KD, H], BF16,
                           kind="ExternalInput") if anyb else None)
    # 4 bias groups packed per partition: [b_f | b_i | b_h | b_h + 0.5]
    bias_d = nc.dram_tensor("biasp", [P, 4 * HB], F32, kind="ExternalInput")
    h0_d = nc.dram_tensor("h0p", [P, HB], F32, kind="ExternalInput")
    id_d = (nc.dram_tensor("identp", [P, P], BF16, kind="ExternalInput")
            if ts_pe else None)
    ht_d = nc.dram_tensor("ht", [H, T], out_dt, kind="ExternalOutput")

    gg_act_set = _spread(gg_act)
    bb_pool_set = _spread(bb_pool)

    with tile.TileContext(nc) as tc:
        with (
            tc.tile_pool(name="xp", bufs=1) as xp,
            tc.tile_pool(name="wp", bufs=1) as wp,
            tc.tile_pool(name="cp", bufs=1) as cp,
            tc.tile_pool(name="ps", bufs=z_bufs, space="PSUM") as ps,
            tc.tile_pool(name="tsp", bufs=ts_bufs, space="PSUM") as tsp,
            tc.tile_pool(name="gp", bufs=bufs) as gp,
            tc.tile_pool(name="ep", bufs=bufs) as ep,
            tc.tile_pool(name="hop", bufs=hop_bufs) as hop,
        ):
            bias = cp.tile([P, 4 * HB], F32, tag="bias")
            h0 = cp.tile([P, HB], F32, tag="h0")
            ident = (cp.tile([P, P], BF16, tag="ident", name="ident")
                     if ts_pe else None)
            warm = cp.tile([P, 1], F32, tag="warm")

            def load_consts():
                """Emitted after the first matmul-feeding DMAs: bias is
                first needed by sigma_f at ~5us, ident by the first idmm,
                h0 by the first scan -- none on the startup critical path.
                The warm-up sigmoid pre-loads the ACT sigmoid table."""
                nc.sync.dma_start(bias[:], bias_d.ap())
                if ts_pe:
                    nc.sync.dma_start(ident[:], id_d.ap())
                nc.sync.dma_start(h0[:], h0_d.ap())
                nc.scalar.activation(warm[:], h0[:, 0:1], AF.Sigmoid)

            xt8 = (xp.tile([P, 2, 2, T], FP8, tag="x8", name="x8")
                   if any8 else None)
            xtb = (xp.tile([P, KD, T], BF16, tag="xb", name="xb")
                   if anyb else None)
            wt8 = (wp.tile([P, len(g8), 2, 2, H], FP8, tag="w8", name="w8")
                   if any8 else None)
            wtb = (wp.tile([P, len(gb) * KD, H], BF16, tag="wb", name="wb")
                   if anyb else None)

            def load_x(q):
                tsl = slice(q * TQ, (q + 1) * TQ)
                if any8:
                    nc.sync.dma_start(xt8[:, :, :, tsl],
                                      x8_d.ap()[:, :, :, tsl])
                if anyb:
                    nc.sync.dma_start(xtb[:, :, tsl],
                                      xb_d.ap()[:, :, tsl])

            # DMA in fine-grained consumption order so the pipeline fills
            # fast: f-gate fp8 weights + first 512 x columns first (first
            # matmul can start ~2.5us in), then i-gate weights, the rest of
            # quarter 0, the h-gate weights (first needed lag_a2+lag_b units
            # in), and finally quarters 1..3.
            if any8:
                nc.sync.dma_start(wt8[:, 0], w8_d.ap()[:, 0])
                nc.sync.dma_start(xt8[:, :, :, 0:512],
                                  x8_d.ap()[:, :, :, 0:512])
                load_consts()
                nc.sync.dma_start(wt8[:, 1], w8_d.ap()[:, 1])
                nc.sync.dma_start(xt8[:, :, :, 512:TQ],
                                  x8_d.ap()[:, :, :, 512:TQ])
            if anyb:
                if not any8:
                    load_consts()
                nc.sync.dma_start(wtb[:], wb_d.ap())
                nc.sync.dma_start(xtb[:, :, 0:TQ], xb_d.ap()[:, :, 0:TQ])
            for q in range(1, NQ):
                load_x(q)

            def mm_group(z, g, hb, c0, width):
                """z[128, width] (PSUM) = W_g^T[., hb] @ x[., c0:c0+width]."""
                hsl = slice(hb * P, (hb + 1) * P)
                fp8 = gdt[g] == FP8
                nj = 2 if fp8 else KD
                if "mm1" in ablate:
                    nj = 1
                for j in range(nj):
                    for t2 in range(width // 512):
                        c = c0 + t2 * 512
                        zsl = z[:, t2 * 512:(t2 + 1) * 512]
                        if fp8:
                            nc.tensor.matmul(
                                zsl, wt8[:, g8.index(g), j, :, hsl],
                                xt8[:, j, :, c:c + 512],
                                start=(j == 0), stop=(j == nj - 1),
                                perf_mode=mybir.MatmulPerfMode.DoubleRow)
                        else:
                            nc.tensor.matmul(
                                zsl, wtb[:, gb.index(g) * KD + j, hsl],
                                xtb[:, j, c:c + 512],
                                start=(j == 0), stop=(j == nj - 1))

            def gbias(g, hb):
                return bias[:, g * HB + hb:g * HB + hb + 1]

            def stage_a(c0, w, hb):
                """zf/zi matmuls -> sigmoids (bf16 SBUF)."""
                tf = gp.tile([P, w], BF16, tag="tf")
                ti = gp.tile([P, w], BF16, tag="ti")
                for g, gt in ((0, tf), (1, ti)):
                    z = ps.tile([P, w], F32, tag="z")
                    if "mm" not in ablate:
                        mm_group(z, g, hb, c0, w)
                    nc.scalar.activation(gt[:], z[:], AF.Sigmoid,
                                         bias=gbias(g, hb))
                return (c0, w, hb, tf, ti)

            def stage_a2(st):
                """ts (PE idmm) -> rr (DVE recip) -> qq (Pool) -> aa (DVE)."""
                c0, w, hb, tf, ti = st
                if "nodiv" in ablate:
                    return (c0, w, hb, ti, ti)
                if ts_pe:
                    ts = tsp.tile([P, w], F32, tag="ts")
                    for t2 in range(w // 512):
                        sl = slice(t2 * 512, (t2 + 1) * 512)
                        nc.tensor.matmul(ts[:, sl], ident[:], tf[:, sl],
                                         start=True, stop=False)
                        nc.tensor.matmul(ts[:, sl], ident[:], ti[:, sl],
                                         start=False, stop=True)
                else:
                    ts = ep.tile([P, w], F32, tag="ts")
                    nc.vector.tensor_tensor(ts[:], tf[:], ti[:], ALU.add)
                rr = ep.tile([P, w], F32, tag="rr")
                nc.vector.reciprocal_approx_fast(rr[:], ts[:])
                qq = ep.tile([P, w], BF16, tag="qq")
                getattr(nc, qq_eng).tensor_tensor(qq[:], rr[:], ti[:],
                                                  ALU.mult)
                aa = ep.tile([P, w], BF16, tag="aa")
                if aa_eng == "scalar":
                    nc.scalar.activation(aa[:], qq[:], AF.Identity,
                                         bias=1.0, scale=-1.0)
                else:
                    nc.vector.tensor_scalar(aa[:], qq[:], -1.0, 1.0,
                                            ALU.mult, ALU.add)
                return (c0, w, hb, qq, aa)

            def stage_b1(st, k):
                """zh matmul -> th sigmoid (+ zp identity for gg_act units).

                Keeping the PSUM z tile alive for stage_b2's stt would pin a
                PSUM buffer across an extra pipeline stage, so gg_act units
                materialize zp in SBUF here; the others defer the z+bias add
                to an stt in b2 -- but that stt needs z, so non-gg_act units
                ALSO run their gg stt here, right after th.
                """
                c0, w, hb, qq, aa = st
                z = ps.tile([P, w], F32, tag="z")
                if "mm" not in ablate:
                    mm_group(z, 2, hb, c0, w)
                th = gp.tile([P, w], BF16, tag="th")
                nc.scalar.activation(th[:], z[:], AF.Sigmoid,
                                     bias=gbias(2, hb))
                gg = gp.tile([P, w], BF16, tag="gg")
                if k in gg_act_set:
                    # ACT materializes zp = zh + bh + 0.5; DVE TT max runs in
                    # the 2x bf16 mode (vs 1x for the stt fallback).
                    zp = gp.tile([P, w], BF16, tag="zp")
                    nc.scalar.activation(
                        zp[:], z[:], AF.Identity,
                        bias=bias[:, 3 * HB + hb:3 * HB + hb + 1])
                    nc.vector.tensor_tensor(gg[:], zp[:], th[:], ALU.max)
                else:
                    nc.vector.scalar_tensor_tensor(
                        gg[:], z[:], bias[:, 3 * HB + hb:3 * HB + hb + 1],
                        th[:], ALU.add, ALU.max)
                return (c0, w, hb, qq, aa, gg)

            def stage_b2(st, prev, k):
                """bb -> scan -> out (all inputs >= 1 iteration old)."""
                c0, w, hb, qq, aa, gg = st
                if "nodiv" in ablate:
                    bb = gg
                else:
                    bb = ep.tile([P, w], BF16, tag="bb")
                    beng = nc.gpsimd if k in bb_pool_set else nc.vector
                    beng.tensor_tensor(bb[:], qq[:], gg[:], ALU.mult)

                if "noscan" in ablate:
                    ho = bb
                else:
                    ho = hop.tile([P, w], out_dt, tag=f"ho{hb}")
                    if prev[hb] is None:
                        init = h0[:, hb:hb + 1]
                    else:
                        pho, pw = prev[hb]
                        init = pho[:, pw - 1:pw]
                    nc.vector.tensor_tensor_scan(ho[:], aa[:], bb[:], init,
                                                 ALU.mult, ALU.add)
                    prev[hb] = (ho, w)
                if "nodma" not in ablate:
                    nc.sync.dma_start(
                        ht_d.ap()[hb * P:(hb + 1) * P, c0:c0 + w], ho[:])

            import contextlib
            loop_cm = (tc.For_i(0, loop_reps, 1) if loop_reps
                       else contextlib.nullcontext())
            with loop_cm:
                # Flat unit list; final units split into 512-wide sub-chunks
                # to shorten the dependent tail after the last matmul.
                units = []
                for q in range(NQ):
                    for hb in range(HB):
                        u = q * HB + hb
                        # split head units (pipeline fills at half-unit
                        # granularity while DMAs stream in) and tail units
                        # (shortens the dependent tail after the last mm)
                        subs = (2 if (u >= NQ * HB - tail_units
                                      or u < head_units) else 1)
                        w = TQ // subs
                        for s in range(subs):
                            units.append((q * TQ + s * w, w, hb))
                n = len(units)
                # 4-stage pipeline, emission order per iteration:
                #   b1(k-2): zh matmul + th/zp/gg -- PE/ACT/DVE heads get
                #            work whose inputs are already computed;
                #   b2(k-3): bb/scan/out -- every input >= 1 iteration old,
                #            so the DVE head never waits;
                #   a2(k-1): ts idmm right after zh on PE (sigma(k-1) is an
                #            iteration old), rr/aa at the DVE tail;
                #   a(k):    zf/zi matmuls + sigmoids last.
                prev = [None] * HB
                st_a = {}
                st_a2 = {}
                st_b1 = {}
                b2_lag = 0 if b2_same else 1
                for it in range(n + lag_a2 + lag_b + b2_lag):
                    if not b2_same:
                        kb2 = it - lag_a2 - lag_b - 1
                        if kb2 >= 0:
                            stage_b2(st_b1.pop(kb2), prev, kb2)
                    kb1 = it - lag_a2 - lag_b
                    if 0 <= kb1 < n:
                        st_b1[kb1] = stage_b1(st_a2.pop(kb1), kb1)
                        if b2_same:
                            stage_b2(st_b1.pop(kb1), prev, kb1)
                    if it < n:
                        st_a[it] = stage_a(*units[it])
                    ka = it - lag_a2
                    if 0 <= ka < n:
                        st_a2[ka] = stage_a2(st_a.pop(ka))

    nc.compile()
    return nc


def _get_module():
    if "nc" not in _CACHE:
        _CACHE["nc"] = _build()
    return _CACHE["nc"]


class _Runner:
    """Caches a compiled 8-core shard_map'd PJRT executable of the Bass
    module so repeat kernel() calls skip jax retracing/compilation."""

    def __init__(self, nc):
        import jax
        from jax.experimental.shard_map import shard_map
        from jax.sharding import Mesh, PartitionSpec

        from concourse import bass2jax

        bass2jax.install_neuronx_cc_hook()
        self.nc = nc
        partition_name = (nc.partition_id_tensor.name
                          if nc.partition_id_tensor else None)
        in_names, out_names, out_avals = [], [], []
        for alloc in nc.m.functions[0].allocations:
            if not isinstance(alloc, mybir.MemoryLocationSet):
                continue
            name = alloc.memorylocations[0].name
            if alloc.kind == "ExternalInput":
                if name != partition_name:
                    in_names.append(name)
            elif alloc.kind == "ExternalOutput":
                out_names.append(name)
                out_avals.append(jax.core.ShapedArray(
                    tuple(alloc.tensor_shape), mybir.dt.np(alloc.dtype)))
        self.in_names = in_names
        self.out_names = out_names
        self.out_avals = out_avals
        n_params, n_outs = len(in_names), len(out_names)
        all_names = list(in_names) + list(out_names)
        if partition_name is not None:
            all_names.append(partition_name)

        def _body(*args):
            operands = list(args)
            if partition_name is not None:
                operands.append(bass2jax.partition_id_tensor())
            return tuple(bass2jax._bass_exec_p.bind(
                *operands,
                out_avals=tuple(out_avals),
                in_names=tuple(all_names),
                out_names=tuple(out_names),
                lowering_input_output_aliases=(),
                sim_require_finite=True,
                sim_require_nnan=True,
                nc=nc,
            ))

        devices = jax.devices()[:B]
        mesh = Mesh(np.asarray(devices), ("core",))
        specs = (PartitionSpec("core"),) * (n_params + n_outs)
        out_specs = (PartitionSpec("core"),) * n_outs
        donate = tuple(range(n_params, n_params + n_outs))
        self._jitted = jax.jit(
            shard_map(_body, mesh=mesh, in_specs=specs,
                      out_specs=out_specs, check_rep=False),
            donate_argnums=donate, keep_unused=True)
        self._compiled = None

    def concat_args(self, in_maps):
        concat_in = [
            np.concatenate([np.asarray(m[name]) for m in in_maps], axis=0)
            for name in self.in_names
        ]
        concat_zeros = [
            np.zeros((B * a.shape[0], *a.shape[1:]), a.dtype)
            for a in self.out_avals
        ]
        return concat_in + concat_zeros

    def compiled(self, args):
        if self._compiled is None:
            self._compiled = self._jitted.lower(*args).compile()
        return self._compiled

    def __call__(self, in_maps):
        import jax
        args = self.concat_args(in_maps)
        outs = jax.block_until_ready(self.compiled(args)(*args))
        return [
            {name: np.asarray(outs[i]).reshape(B, *self.out_avals[i].shape)[c]
             for i, name in enumerate(self.out_names)}
            for c in range(B)
        ]


def _get_runner():
    if "runner" not in _CACHE:
        _CACHE["runner"] = _Runner(_get_module())
    return _CACHE["runner"]


def make_in_maps(x, h_0, W_f, b_f, W_i, b_i, W_h, b_h, mm=MM, ts_pe=True):
    gdt = _gate_dtypes(mm)
    any8 = any(d == FP8 for d in gdt.values())
    anyb = any(d == BF16 for d in gdt.values())
    f8 = mybir.dt.np(FP8)
    bf = mybir.dt.np(BF16)
    x = np.asarray(x, np.float32)
    h_0 = np.asarray(h_0, np.float32)
    wT = np.ascontiguousarray(
        np.stack([np.asarray(W_f), np.asarray(W_i), np.asarray(W_h)])
        .astype(np.float32).transpose(0, 2, 1))          # [3, D, H]
    b_h = np.asarray(b_h)
    biasp = np.ascontiguousarray(
        np.stack([np.asarray(b_f), np.asarray(b_i), b_h, b_h + 0.5])
        .astype(np.float32).reshape(4, HB, P).transpose(2, 0, 1)
        .reshape(P, 4 * HB))
    g8 = [g for g in range(3) if gdt[g] == FP8]
    gb = [g for g in range(3) if gdt[g] == BF16]
    common = {"biasp": biasp}
    if ts_pe:
        common["identp"] = np.eye(P, dtype=bf)
    if any8:
        # [3, D, H] -> [p, gi, j, i, H] with d = (2j+i)*128+p
        w8 = wT[g8].reshape(len(g8), 2, 2, P, H)         # [gi, j, i, p, H]
        common["wT8"] = np.ascontiguousarray(
            w8.transpose(3, 0, 1, 2, 4)).astype(f8)
    if anyb:
        # [3, D, H] -> [p, ki, H] with d = k*128+p, ki = gb_idx*KD + k
        wb = wT[gb].reshape(len(gb), KD, P, H)           # [gi, k, p, H]
        common["wTb"] = np.ascontiguousarray(
            wb.transpose(2, 0, 1, 3).reshape(P, len(gb) * KD, H)).astype(bf)
    in_maps = []
    for b in range(B):
        xT = np.ascontiguousarray(x[b].T)                # [D, T]
        m = dict(common)
        if any8:
            # [D, T] -> [p, j, i, T]
            m["xT8"] = np.ascontiguousarray(
                xT.reshape(2, 2, P, T).transpose(2, 0, 1, 3)).astype(f8)
        if anyb:
            # [D, T] -> [p, k, T]
            m["xTb"] = np.ascontiguousarray(
                xT.reshape(KD, P, T).transpose(1, 0, 2)).astype(bf)
        m["h0p"] = np.ascontiguousarray(h_0[b].reshape(HB, P).T)
        in_maps.append(m)
    return in_maps


def kernel(x, h_0, W_f, b_f, W_i, b_i, W_h, b_h):
    in_maps = make_in_maps(x, h_0, W_f, b_f, W_i, b_i, W_h, b_h)
    results = _get_runner()(in_maps)
    out = np.empty((B, T, H), np.float32)
    for b in range(B):
        out[b] = results[b]["ht"].astype(np.float32).T
    return out
